# revision 25
# baseline (speedup 1.0000x reference)
"""Trainium2 Bass kernel for the E67H gated recurrent cell.

  alpha_x = x @ W_alpha.T ; v = tanh(x @ W_x.T + b_v)
  alpha_t = sigmoid(alpha_x_t + d*h_{t-1} + b_alpha)
  h_t     = alpha_t*h_{t-1} + (1-alpha_t)*v_t
  out_t   = h_t^2 * sigmoid(h_t)

Strategy: data-parallel over batch B=32 across 8 NeuronCores (4 batch rows
per core); weights replicated; no collectives.

Per-core algorithm (feature-major layout, tiles [128 partitions = feature
slice, free = (t, chunk, batch)]):

With p_t = h_t - v_{t+1} (v_{-1} := h0, m_{-1} := 0):
  m_t   = alpha_t * (m_{t-1} + w_{t-1}),  w_{t-1} = v_{t-1} - v_t
  h_t   = v_t + m_t
  u_t   = ax_t + b_alpha + d*v_{t-1} + d*m_{t-1}   (sigmoid logit)
In d-scaled space m~ = d*m, w~ = d*w the sequential chain per step is just
  u = C~_t + m~_{t-1};  alpha = sigmoid(u);  m~_t = alpha*(m~_{t-1}+w~_{t-1})
with C~_t = ax_t + b_alpha + d*v_{t-1} and w~ bulk-precomputed from the
GEMMs.  Outputs are reconstructed in bulk: h = v + m~ * (1/d).

d is clamped away from 0 on the host (|d|<1e-12 -> 1e-12), which perturbs
the logit by <=1e-12*|h| — far below fp32 noise.
"""

import os
from contextlib import ExitStack

import numpy as np
import ml_dtypes

import concourse.bass as bass
import concourse.tile as tile
from concourse import bacc, mybir
from concourse.bass_utils import run_bass_kernel_spmd

T, B, D = 1024, 32, 1024
NCORES = 8
BL = B // NCORES          # batch rows per core
P = 128                   # partitions
C = D // P                # feature chunks per core
F32 = mybir.dt.float32
BF16 = mybir.dt.bfloat16
AF = mybir.ActivationFunctionType
OP = mybir.AluOpType


def build_kernel(Tk=T, W=64):
    """Build the per-core Bass kernel. Tk = sequence length, W = window."""
    assert Tk % W == 0
    if Tk == 1024 and W == 64:
        # small first window so the recurrence starts ~20us earlier
        WLIST = [16, 48] + [64] * 15
    else:
        WLIST = [W] * (Tk // W)
    T0S = [sum(WLIST[:i]) for i in range(len(WLIST))]
    NW = len(WLIST)
    S = C * BL                     # free columns per step (32)

    nc = bacc.Bacc(None)

    xT = nc.declare_dram_parameter("xT", [D, Tk * BL], BF16, isOutput=False)
    WaT = nc.declare_dram_parameter("WaT", [D, D], BF16, isOutput=False)
    WxT = nc.declare_dram_parameter("WxT", [D, D], BF16, isOutput=False)
    h0 = nc.declare_dram_parameter("h0", [BL, D], F32, isOutput=False)
    dvec = nc.declare_dram_parameter("dvec", [D], F32, isOutput=False)
    dinv = nc.declare_dram_parameter("dinv", [D], F32, isOutput=False)
    dneg = nc.declare_dram_parameter("dneg", [D], F32, isOutput=False)
    bav = nc.declare_dram_parameter("bav", [D], F32, isOutput=False)
    bvv = nc.declare_dram_parameter("bvv", [D], F32, isOutput=False)
    hs_out = nc.declare_dram_parameter("hs", [P, Tk, C, BL], F32, isOutput=True)
    outs_out = nc.declare_dram_parameter("outs", [P, Tk, C, BL], F32, isOutput=True)

    with tile.TileContext(nc) as tc, ExitStack() as ctx:
        consts = ctx.enter_context(tc.tile_pool(name="consts", bufs=1))
        xw_pool = ctx.enter_context(tc.tile_pool(name="xw", bufs=3))
        vw_pool = ctx.enter_context(tc.tile_pool(name="vw", bufs=3))
        dvs_pool = ctx.enter_context(tc.tile_pool(name="dvs", bufs=2))
        cw_pool = ctx.enter_context(tc.tile_pool(name="cw", bufs=2))
        wt_pool = ctx.enter_context(tc.tile_pool(name="wt", bufs=2))
        mt_pool = ctx.enter_context(tc.tile_pool(name="mt", bufs=3))
        small = ctx.enter_context(tc.tile_pool(name="small", bufs=8))
        hw_pool = ctx.enter_context(tc.tile_pool(name="hw", bufs=2))
        sq_pool = ctx.enter_context(tc.tile_pool(name="sq", bufs=2))
        sg_pool = ctx.enter_context(tc.tile_pool(name="sg", bufs=2))
        ow_pool = ctx.enter_context(tc.tile_pool(name="ow", bufs=2))
        psum = ctx.enter_context(
            tc.tile_pool(name="psum", bufs=6, space="PSUM")
        )

        # ---- constants (W_x and xw(0) first: they gate the first GEMM;
        # W_a is not needed until the first ax-GEMM) ----
        Wx_sb = consts.tile([P, C, D], BF16)
        nc.sync.dma_start(out=Wx_sb, in_=WxT.rearrange("(kc p) e -> p kc e", p=P))

        d_pc = consts.tile([P, C], F32)
        nc.sync.dma_start(out=d_pc, in_=dvec.rearrange("(c p) -> p c", p=P))
        dinv_pc = consts.tile([P, C], F32)
        nc.sync.dma_start(out=dinv_pc, in_=dinv.rearrange("(c p) -> p c", p=P))
        dneg_pc = consts.tile([P, C], F32)
        nc.sync.dma_start(out=dneg_pc, in_=dneg.rearrange("(c p) -> p c", p=P))
        ba_pc = consts.tile([P, C], F32)
        nc.sync.dma_start(out=ba_pc, in_=bav.rearrange("(c p) -> p c", p=P))
        bv_pc = consts.tile([P, C], F32)
        nc.sync.dma_start(out=bv_pc, in_=bvv.rearrange("(c p) -> p c", p=P))
        h0_sb = consts.tile([P, C, BL], F32)
        for c in range(C):
            nc.sync.dma_start(
                out=h0_sb[:, c, :],
                in_=h0[:, c * P : (c + 1) * P].rearrange("b p -> p b"),
            )

        mzero = consts.tile([P, S], F32)
        nc.vector.memset(mzero, 0.0)

        v_tiles = [None] * NW
        dvs_tiles = [None] * NW
        mt_tiles = [None] * NW

        def load_xw(w):
            t0, Wc = T0S[w], WLIST[w]
            xw = xw_pool.tile([P, C, Wc * BL], BF16)
            nc.sync.dma_start(
                out=xw,
                in_=xT.rearrange("(kc p) n -> p kc n", p=P)[
                    :, :, t0 * BL : (t0 + Wc) * BL
                ],
            )
            return xw

        xw_tiles = [None] * NW
        xw_tiles[0] = load_xw(0)

        Wa_sb = consts.tile([P, C, D], BF16)
        nc.sync.dma_start(out=Wa_sb, in_=WaT.rearrange("(kc p) e -> p kc e", p=P))

        def emit_post(w):
            """Post-pass + output DMAs for window w (emitted one window late
            so they fill chain bubbles instead of blocking the boundary)."""
            t0, W = T0S[w], WLIST[w]
            mt = mt_tiles[w]
            vw = v_tiles[w]
            hw = hw_pool.tile([P, W, C, BL], F32)
            for ec in range(C):
                for hh in range(2):
                    t_lo, t_hi = hh * (W // 2), (hh + 1) * (W // 2)
                    nc.vector.scalar_tensor_tensor(
                        out=hw[:, t_lo:t_hi, ec, :],
                        in0=mt[:, t_lo:t_hi, ec, :],
                        scalar=dinv_pc[:, ec : ec + 1],
                        in1=vw[:, t_lo:t_hi, ec, :],
                        op0=OP.mult,
                        op1=OP.add,
                    )
            sg = sg_pool.tile([P, W, C, BL], F32)
            sq = sq_pool.tile([P, W, C, BL], F32)
            ow = ow_pool.tile([P, W, C, BL], F32)
            QT = max(1, W // 8)
            for q in range((W + QT - 1) // QT):
                lo, hi = q * QT, min((q + 1) * QT, W)
                nc.scalar.activation(
                    out=sg[:, lo:hi], in_=hw[:, lo:hi], func=AF.Sigmoid
                )
                nc.scalar.activation(
                    out=sq[:, lo:hi], in_=hw[:, lo:hi], func=AF.Square
                )
                nc.vector.tensor_mul(ow[:, lo:hi], sq[:, lo:hi], sg[:, lo:hi])
            nc.sync.dma_start(out=hs_out[:, t0 : t0 + W, :, :], in_=hw)
            nc.sync.dma_start(out=outs_out[:, t0 : t0 + W, :, :], in_=ow)

        for w in range(NW):
            t0, W = T0S[w], WLIST[w]
            if w + 1 < NW:
                xw_tiles[w + 1] = load_xw(w + 1)
            xw = xw_tiles[w]

            # ---- v GEMM: v = tanh(x W_x^T + b_v) ----
            vw = vw_pool.tile([P, W, C, BL], F32)
            v_tiles[w] = vw
            for ec in range(C):
                ps = psum.tile([P, W * BL], F32)
                for kc in range(C):
                    nc.tensor.matmul(
                        ps,
                        Wx_sb[:, kc, ec * P : (ec + 1) * P],
                        xw[:, kc, :],
                        start=(kc == 0),
                        stop=(kc == C - 1),
                    )
                n_tanh = 2
                for hh in range(n_tanh):
                    t_lo, t_hi = hh * (W // n_tanh), (hh + 1) * (W // n_tanh)
                    nc.scalar.activation(
                        out=vw[:, t_lo:t_hi, ec, :],
                        in_=ps[:, t_lo * BL : t_hi * BL],
                        func=AF.Tanh,
                        bias=bv_pc[:, ec : ec + 1],
                    )

            # ---- dvs window: slot s holds d*v_{t0+s-1}  (pure, no bias) ----
            dvs = dvs_pool.tile([P, W, C, BL], F32)
            dvs_tiles[w] = dvs
            for ec in range(C):
                if w == 0:
                    nc.vector.tensor_scalar_mul(
                        dvs[:, 0, ec, :], h0_sb[:, ec, :], d_pc[:, ec : ec + 1]
                    )
                else:
                    nc.vector.tensor_scalar_mul(
                        dvs[:, 0, ec, :],
                        v_tiles[w - 1][:, WLIST[w - 1] - 1, ec, :],
                        d_pc[:, ec : ec + 1],
                    )
                for (a, b) in ((1, W // 2), (W // 2, W)):
                    nc.vector.tensor_scalar_mul(
                        dvs[:, a:b, ec, :],
                        vw[:, a - 1 : b - 1, ec, :],
                        d_pc[:, ec : ec + 1],
                    )

            # ---- w~ window: slot s = dvs[s] - d*v_{t0+s} ----
            wt = wt_pool.tile([P, W, C, BL], F32)
            QW = max(1, W // 16)
            for q in range((W + QW - 1) // QW):
                lo = q * QW
                hi = min((q + 1) * QW, W - 1)
                if hi > lo:
                    nc.vector.tensor_sub(
                        wt[:, lo:hi, :, :],
                        dvs[:, lo:hi, :, :],
                        dvs[:, lo + 1 : hi + 1, :, :],
                    )
            for ec in range(C):
                # w~[W-1] = dvs[W-1] + (-d)*v[W-1]
                nc.vector.scalar_tensor_tensor(
                    out=wt[:, W - 1, ec, :],
                    in0=vw[:, W - 1, ec, :],
                    scalar=dneg_pc[:, ec : ec + 1],
                    in1=dvs[:, W - 1, ec, :],
                    op0=OP.mult,
                    op1=OP.add,
                )

            # ---- ax GEMM and C~ = (ax + b_alpha) + dvs ----
            cw = cw_pool.tile([P, W, C, BL], F32)
            for ec in range(C):
                ps = psum.tile([P, W * BL], F32)
                for kc in range(C):
                    nc.tensor.matmul(
                        ps,
                        Wa_sb[:, kc, ec * P : (ec + 1) * P],
                        xw[:, kc, :],
                        start=(kc == 0),
                        stop=(kc == C - 1),
                    )
                for hh in range(4):
                    t_lo, t_hi = hh * (W // 4), (hh + 1) * (W // 4)
                    nc.vector.scalar_tensor_tensor(
                        out=cw[:, t_lo:t_hi, ec, :],
                        in0=ps[:, t_lo * BL : t_hi * BL],
                        scalar=ba_pc[:, ec : ec + 1],
                        in1=dvs[:, t_lo:t_hi, ec, :],
                        op0=OP.add,
                        op1=OP.add,
                    )

            # ---- sequential recurrence over the window ----
            mt = mt_pool.tile([P, W, C, BL], F32)
            mt_tiles[w] = mt
            for s in range(W):
                if w == 0 and s == 0:
                    mprev = mzero[:, :]
                elif s == 0:
                    mprev = mt_tiles[w - 1][:, WLIST[w - 1] - 1, :, :].rearrange(
                        "p c b -> p (c b)"
                    )
                else:
                    mprev = mt[:, s - 1, :, :]
                u = small.tile([P, S], F32)
                nc.vector.tensor_add(
                    u, cw[:, s, :, :], mprev
                )
                al = small.tile([P, S], F32)
                nc.scalar.activation(out=al, in_=u, func=AF.Sigmoid)
                g = small.tile([P, S], F32)
                nc.vector.tensor_add(
                    g, mprev, wt[:, s, :, :]
                )
                nc.vector.tensor_mul(
                    mt[:, s, :, :], al, g
                )

            # ---- deferred post-pass of the previous window ----
            if w > 0:
                emit_post(w - 1)

        emit_post(NW - 1)

    nc.compile()
    return nc


_BUILD_CACHE = {}


def _get_nc(Tk=T, W=64):
    key = (Tk, W)
    if key not in _BUILD_CACHE:
        _BUILD_CACHE[key] = build_kernel(Tk, W)
    return _BUILD_CACHE[key]


LAST_EXEC_NS = None


def kernel(x, h0, W_alpha, d_alpha, b_alpha, W_x, b_v):
    """Full-input entry point. Returns (outs, h) like the reference."""
    global LAST_EXEC_NS
    x = np.asarray(x, dtype=np.float32)
    h0 = np.asarray(h0, dtype=np.float32)
    W_alpha = np.asarray(W_alpha, dtype=np.float32)
    d_alpha = np.asarray(d_alpha, dtype=np.float32)
    b_alpha = np.asarray(b_alpha, dtype=np.float32)
    W_x = np.asarray(W_x, dtype=np.float32)
    b_v = np.asarray(b_v, dtype=np.float32)

    Tk, Bk, Dk = x.shape
    assert (Bk, Dk) == (B, D)

    d_c = np.where(np.abs(d_alpha) < 1e-12, np.float32(1e-12), d_alpha).astype(
        np.float32
    )
    dinv = (np.float32(1.0) / d_c).astype(np.float32)

    WaT = np.ascontiguousarray(W_alpha.T).astype(ml_dtypes.bfloat16)
    WxT = np.ascontiguousarray(W_x.T).astype(ml_dtypes.bfloat16)

    W_win = 64
    nc = _get_nc(Tk, W_win)

    in_maps = []
    for i in range(NCORES):
        xs = x[:, i * BL : (i + 1) * BL, :]                   # [T, BL, D]
        xTl = np.ascontiguousarray(xs.transpose(2, 0, 1)).reshape(D, Tk * BL)
        in_maps.append(
            {
                "xT": xTl.astype(ml_dtypes.bfloat16),
                "WaT": WaT,
                "WxT": WxT,
                "h0": np.ascontiguousarray(h0[i * BL : (i + 1) * BL, :]),
                "dvec": d_c,
                "dinv": dinv,
                "dneg": (-d_c).astype(np.float32),
                "bav": b_alpha,
                "bvv": b_v,
            }
        )

    res = run_bass_kernel_spmd(
        nc, in_maps, core_ids=list(range(NCORES)), trace=False
    )
    LAST_EXEC_NS = res.exec_time_ns

    outs = np.empty((Tk, B, D), dtype=np.float32)
    h = np.empty((Tk + 1, B, D), dtype=np.float32)
    h[0] = h0
    for i in range(NCORES):
        r = res.results[i]
        # [P, Tk, C, BL] -> [Tk, BL, C, P] -> [Tk, BL, D]
        hs_i = np.ascontiguousarray(np.transpose(r["hs"], (1, 3, 2, 0))).reshape(
            Tk, BL, D
        )
        outs_i = np.ascontiguousarray(
            np.transpose(r["outs"], (1, 3, 2, 0))
        ).reshape(Tk, BL, D)
        h[1:, i * BL : (i + 1) * BL, :] = hs_i
        outs[:, i * BL : (i + 1) * BL, :] = outs_i
    return outs, h


# revision 26
# speedup vs baseline: 1.0029x; 1.0029x over previous
"""Trainium2 Bass kernel for the E67H gated recurrent cell.

  alpha_x = x @ W_alpha.T ; v = tanh(x @ W_x.T + b_v)
  alpha_t = sigmoid(alpha_x_t + d*h_{t-1} + b_alpha)
  h_t     = alpha_t*h_{t-1} + (1-alpha_t)*v_t
  out_t   = h_t^2 * sigmoid(h_t)

Strategy: data-parallel over batch B=32 across 8 NeuronCores (4 batch rows
per core); weights replicated; no collectives.

Per-core algorithm (feature-major layout, tiles [128 partitions = feature
slice, free = (t, chunk, batch)]):

With p_t = h_t - v_{t+1} (v_{-1} := h0, m_{-1} := 0):
  m_t   = alpha_t * (m_{t-1} + w_{t-1}),  w_{t-1} = v_{t-1} - v_t
  h_t   = v_t + m_t
  u_t   = ax_t + b_alpha + d*v_{t-1} + d*m_{t-1}   (sigmoid logit)
In d-scaled space m~ = d*m, w~ = d*w the sequential chain per step is just
  u = C~_t + m~_{t-1};  alpha = sigmoid(u);  m~_t = alpha*(m~_{t-1}+w~_{t-1})
with C~_t = ax_t + b_alpha + d*v_{t-1} and w~ bulk-precomputed from the
GEMMs.  Outputs are reconstructed in bulk: h = v + m~ * (1/d).

d is clamped away from 0 on the host (|d|<1e-12 -> 1e-12), which perturbs
the logit by <=1e-12*|h| — far below fp32 noise.
"""

import os
from contextlib import ExitStack

import numpy as np
import ml_dtypes

import concourse.bass as bass
import concourse.tile as tile
from concourse import bacc, mybir
from concourse.bass_utils import run_bass_kernel_spmd

T, B, D = 1024, 32, 1024
NCORES = 8
BL = B // NCORES          # batch rows per core
P = 128                   # partitions
C = D // P                # feature chunks per core
F32 = mybir.dt.float32
BF16 = mybir.dt.bfloat16
AF = mybir.ActivationFunctionType
OP = mybir.AluOpType


def build_kernel(Tk=T, W=64):
    """Build the per-core Bass kernel. Tk = sequence length, W = window."""
    assert Tk % W == 0
    if Tk == 1024 and W == 64:
        # small first window so the recurrence starts ~20us earlier
        WLIST = [16, 48] + [64] * 15
    else:
        WLIST = [W] * (Tk // W)
    T0S = [sum(WLIST[:i]) for i in range(len(WLIST))]
    NW = len(WLIST)
    S = C * BL                     # free columns per step (32)

    nc = bacc.Bacc(None)

    xT = nc.declare_dram_parameter("xT", [D, Tk * BL], BF16, isOutput=False)
    WaT = nc.declare_dram_parameter("WaT", [D, D], BF16, isOutput=False)
    WxT = nc.declare_dram_parameter("WxT", [D, D], BF16, isOutput=False)
    h0 = nc.declare_dram_parameter("h0", [BL, D], F32, isOutput=False)
    dvec = nc.declare_dram_parameter("dvec", [D], F32, isOutput=False)
    dinv = nc.declare_dram_parameter("dinv", [D], F32, isOutput=False)
    dneg = nc.declare_dram_parameter("dneg", [D], F32, isOutput=False)
    bav = nc.declare_dram_parameter("bav", [D], F32, isOutput=False)
    bvv = nc.declare_dram_parameter("bvv", [D], F32, isOutput=False)
    hs_out = nc.declare_dram_parameter("hs", [P, Tk, C, BL], F32, isOutput=True)
    outs_out = nc.declare_dram_parameter("outs", [P, Tk, C, BL], F32, isOutput=True)

    with tile.TileContext(nc) as tc, ExitStack() as ctx:
        consts = ctx.enter_context(tc.tile_pool(name="consts", bufs=1))
        xw_pool = ctx.enter_context(tc.tile_pool(name="xw", bufs=3))
        vw_pool = ctx.enter_context(tc.tile_pool(name="vw", bufs=3))
        dvs_pool = ctx.enter_context(tc.tile_pool(name="dvs", bufs=2))
        cw_pool = ctx.enter_context(tc.tile_pool(name="cw", bufs=2))
        wt_pool = ctx.enter_context(tc.tile_pool(name="wt", bufs=2))
        mt_pool = ctx.enter_context(tc.tile_pool(name="mt", bufs=3))
        small = ctx.enter_context(tc.tile_pool(name="small", bufs=8))
        hw_pool = ctx.enter_context(tc.tile_pool(name="hw", bufs=2))
        sq_pool = ctx.enter_context(tc.tile_pool(name="sq", bufs=2))
        sg_pool = ctx.enter_context(tc.tile_pool(name="sg", bufs=2))
        ow_pool = ctx.enter_context(tc.tile_pool(name="ow", bufs=2))
        psum = ctx.enter_context(
            tc.tile_pool(name="psum", bufs=8, space="PSUM")
        )

        # ---- constants (W_x and xw(0) first: they gate the first GEMM;
        # W_a is not needed until the first ax-GEMM) ----
        Wx_sb = consts.tile([P, C, D], BF16)
        nc.sync.dma_start(out=Wx_sb, in_=WxT.rearrange("(kc p) e -> p kc e", p=P))

        d_pc = consts.tile([P, C], F32)
        nc.sync.dma_start(out=d_pc, in_=dvec.rearrange("(c p) -> p c", p=P))
        dinv_pc = consts.tile([P, C], F32)
        nc.sync.dma_start(out=dinv_pc, in_=dinv.rearrange("(c p) -> p c", p=P))
        dneg_pc = consts.tile([P, C], F32)
        nc.sync.dma_start(out=dneg_pc, in_=dneg.rearrange("(c p) -> p c", p=P))
        ba_pc = consts.tile([P, C], F32)
        nc.sync.dma_start(out=ba_pc, in_=bav.rearrange("(c p) -> p c", p=P))
        bv_pc = consts.tile([P, C], F32)
        nc.sync.dma_start(out=bv_pc, in_=bvv.rearrange("(c p) -> p c", p=P))
        h0_sb = consts.tile([P, C, BL], F32)
        for c in range(C):
            nc.sync.dma_start(
                out=h0_sb[:, c, :],
                in_=h0[:, c * P : (c + 1) * P].rearrange("b p -> p b"),
            )

        mzero = consts.tile([P, S], F32)
        nc.vector.memset(mzero, 0.0)

        v_tiles = [None] * NW
        dvs_tiles = [None] * NW
        mt_tiles = [None] * NW

        def load_xw(w):
            t0, Wc = T0S[w], WLIST[w]
            xw = xw_pool.tile([P, C, Wc * BL], BF16)
            nc.sync.dma_start(
                out=xw,
                in_=xT.rearrange("(kc p) n -> p kc n", p=P)[
                    :, :, t0 * BL : (t0 + Wc) * BL
                ],
            )
            return xw

        xw_tiles = [None] * NW
        xw_tiles[0] = load_xw(0)

        Wa_sb = consts.tile([P, C, D], BF16)
        nc.sync.dma_start(out=Wa_sb, in_=WaT.rearrange("(kc p) e -> p kc e", p=P))

        def emit_post(w):
            """Post-pass + output DMAs for window w (emitted one window late
            so they fill chain bubbles instead of blocking the boundary)."""
            t0, W = T0S[w], WLIST[w]
            mt = mt_tiles[w]
            vw = v_tiles[w]
            hw = hw_pool.tile([P, W, C, BL], F32)
            for ec in range(C):
                for hh in range(2):
                    t_lo, t_hi = hh * (W // 2), (hh + 1) * (W // 2)
                    nc.vector.scalar_tensor_tensor(
                        out=hw[:, t_lo:t_hi, ec, :],
                        in0=mt[:, t_lo:t_hi, ec, :],
                        scalar=dinv_pc[:, ec : ec + 1],
                        in1=vw[:, t_lo:t_hi, ec, :],
                        op0=OP.mult,
                        op1=OP.add,
                    )
            sg = sg_pool.tile([P, W, C, BL], F32)
            sq = sq_pool.tile([P, W, C, BL], F32)
            ow = ow_pool.tile([P, W, C, BL], F32)
            QT = max(1, W // 8)
            for q in range((W + QT - 1) // QT):
                lo, hi = q * QT, min((q + 1) * QT, W)
                nc.scalar.activation(
                    out=sg[:, lo:hi], in_=hw[:, lo:hi], func=AF.Sigmoid
                )
                nc.scalar.activation(
                    out=sq[:, lo:hi], in_=hw[:, lo:hi], func=AF.Square
                )
                nc.vector.tensor_mul(ow[:, lo:hi], sq[:, lo:hi], sg[:, lo:hi])
            nc.sync.dma_start(out=hs_out[:, t0 : t0 + W, :, :], in_=hw)
            nc.sync.dma_start(out=outs_out[:, t0 : t0 + W, :, :], in_=ow)

        for w in range(NW):
            t0, W = T0S[w], WLIST[w]
            if w + 1 < NW:
                xw_tiles[w + 1] = load_xw(w + 1)
            xw = xw_tiles[w]

            # ---- v GEMM: v = tanh(x W_x^T + b_v) ----
            vw = vw_pool.tile([P, W, C, BL], F32)
            v_tiles[w] = vw
            for ec in range(C):
                ps = psum.tile([P, W * BL], F32)
                for kc in range(C):
                    nc.tensor.matmul(
                        ps,
                        Wx_sb[:, kc, ec * P : (ec + 1) * P],
                        xw[:, kc, :],
                        start=(kc == 0),
                        stop=(kc == C - 1),
                    )
                n_tanh = 2
                for hh in range(n_tanh):
                    t_lo, t_hi = hh * (W // n_tanh), (hh + 1) * (W // n_tanh)
                    nc.scalar.activation(
                        out=vw[:, t_lo:t_hi, ec, :],
                        in_=ps[:, t_lo * BL : t_hi * BL],
                        func=AF.Tanh,
                        bias=bv_pc[:, ec : ec + 1],
                    )

            # ---- dvs window: slot s holds d*v_{t0+s-1}  (pure, no bias) ----
            dvs = dvs_pool.tile([P, W, C, BL], F32)
            dvs_tiles[w] = dvs
            for ec in range(C):
                if w == 0:
                    nc.vector.tensor_scalar_mul(
                        dvs[:, 0, ec, :], h0_sb[:, ec, :], d_pc[:, ec : ec + 1]
                    )
                else:
                    nc.vector.tensor_scalar_mul(
                        dvs[:, 0, ec, :],
                        v_tiles[w - 1][:, WLIST[w - 1] - 1, ec, :],
                        d_pc[:, ec : ec + 1],
                    )
                for (a, b) in ((1, W // 2), (W // 2, W)):
                    nc.vector.tensor_scalar_mul(
                        dvs[:, a:b, ec, :],
                        vw[:, a - 1 : b - 1, ec, :],
                        d_pc[:, ec : ec + 1],
                    )

            # ---- w~ window: slot s = dvs[s] - d*v_{t0+s} ----
            wt = wt_pool.tile([P, W, C, BL], F32)
            QW = max(1, W // 16)
            for q in range((W + QW - 1) // QW):
                lo = q * QW
                hi = min((q + 1) * QW, W - 1)
                if hi > lo:
                    nc.vector.tensor_sub(
                        wt[:, lo:hi, :, :],
                        dvs[:, lo:hi, :, :],
                        dvs[:, lo + 1 : hi + 1, :, :],
                    )
            for ec in range(C):
                # w~[W-1] = dvs[W-1] + (-d)*v[W-1]
                nc.vector.scalar_tensor_tensor(
                    out=wt[:, W - 1, ec, :],
                    in0=vw[:, W - 1, ec, :],
                    scalar=dneg_pc[:, ec : ec + 1],
                    in1=dvs[:, W - 1, ec, :],
                    op0=OP.mult,
                    op1=OP.add,
                )

            # ---- ax GEMM and C~ = (ax + b_alpha) + dvs ----
            cw = cw_pool.tile([P, W, C, BL], F32)
            for ec in range(C):
                ps = psum.tile([P, W * BL], F32)
                for kc in range(C):
                    nc.tensor.matmul(
                        ps,
                        Wa_sb[:, kc, ec * P : (ec + 1) * P],
                        xw[:, kc, :],
                        start=(kc == 0),
                        stop=(kc == C - 1),
                    )
                for hh in range(4):
                    t_lo, t_hi = hh * (W // 4), (hh + 1) * (W // 4)
                    nc.vector.scalar_tensor_tensor(
                        out=cw[:, t_lo:t_hi, ec, :],
                        in0=ps[:, t_lo * BL : t_hi * BL],
                        scalar=ba_pc[:, ec : ec + 1],
                        in1=dvs[:, t_lo:t_hi, ec, :],
                        op0=OP.add,
                        op1=OP.add,
                    )

            # ---- sequential recurrence over the window ----
            mt = mt_pool.tile([P, W, C, BL], F32)
            mt_tiles[w] = mt
            for s in range(W):
                if w == 0 and s == 0:
                    mprev = mzero[:, :]
                elif s == 0:
                    mprev = mt_tiles[w - 1][:, WLIST[w - 1] - 1, :, :].rearrange(
                        "p c b -> p (c b)"
                    )
                else:
                    mprev = mt[:, s - 1, :, :]
                u = small.tile([P, S], F32)
                nc.vector.tensor_add(
                    u, cw[:, s, :, :], mprev
                )
                al = small.tile([P, S], F32)
                nc.scalar.activation(out=al, in_=u, func=AF.Sigmoid)
                g = small.tile([P, S], F32)
                nc.vector.tensor_add(
                    g, mprev, wt[:, s, :, :]
                )
                nc.vector.tensor_mul(
                    mt[:, s, :, :], al, g
                )

            # ---- deferred post-pass of the previous window ----
            if w > 0:
                emit_post(w - 1)

        emit_post(NW - 1)

    nc.compile()
    return nc


_BUILD_CACHE = {}


def _get_nc(Tk=T, W=64):
    key = (Tk, W)
    if key not in _BUILD_CACHE:
        _BUILD_CACHE[key] = build_kernel(Tk, W)
    return _BUILD_CACHE[key]


LAST_EXEC_NS = None


def kernel(x, h0, W_alpha, d_alpha, b_alpha, W_x, b_v):
    """Full-input entry point. Returns (outs, h) like the reference."""
    global LAST_EXEC_NS
    x = np.asarray(x, dtype=np.float32)
    h0 = np.asarray(h0, dtype=np.float32)
    W_alpha = np.asarray(W_alpha, dtype=np.float32)
    d_alpha = np.asarray(d_alpha, dtype=np.float32)
    b_alpha = np.asarray(b_alpha, dtype=np.float32)
    W_x = np.asarray(W_x, dtype=np.float32)
    b_v = np.asarray(b_v, dtype=np.float32)

    Tk, Bk, Dk = x.shape
    assert (Bk, Dk) == (B, D)

    d_c = np.where(np.abs(d_alpha) < 1e-12, np.float32(1e-12), d_alpha).astype(
        np.float32
    )
    dinv = (np.float32(1.0) / d_c).astype(np.float32)

    WaT = np.ascontiguousarray(W_alpha.T).astype(ml_dtypes.bfloat16)
    WxT = np.ascontiguousarray(W_x.T).astype(ml_dtypes.bfloat16)

    W_win = 64
    nc = _get_nc(Tk, W_win)

    in_maps = []
    for i in range(NCORES):
        xs = x[:, i * BL : (i + 1) * BL, :]                   # [T, BL, D]
        xTl = np.ascontiguousarray(xs.transpose(2, 0, 1)).reshape(D, Tk * BL)
        in_maps.append(
            {
                "xT": xTl.astype(ml_dtypes.bfloat16),
                "WaT": WaT,
                "WxT": WxT,
                "h0": np.ascontiguousarray(h0[i * BL : (i + 1) * BL, :]),
                "dvec": d_c,
                "dinv": dinv,
                "dneg": (-d_c).astype(np.float32),
                "bav": b_alpha,
                "bvv": b_v,
            }
        )

    res = run_bass_kernel_spmd(
        nc, in_maps, core_ids=list(range(NCORES)), trace=False
    )
    LAST_EXEC_NS = res.exec_time_ns

    outs = np.empty((Tk, B, D), dtype=np.float32)
    h = np.empty((Tk + 1, B, D), dtype=np.float32)
    h[0] = h0
    for i in range(NCORES):
        r = res.results[i]
        # [P, Tk, C, BL] -> [Tk, BL, C, P] -> [Tk, BL, D]
        hs_i = np.ascontiguousarray(np.transpose(r["hs"], (1, 3, 2, 0))).reshape(
            Tk, BL, D
        )
        outs_i = np.ascontiguousarray(
            np.transpose(r["outs"], (1, 3, 2, 0))
        ).reshape(Tk, BL, D)
        h[1:, i * BL : (i + 1) * BL, :] = hs_i
        outs[:, i * BL : (i + 1) * BL, :] = outs_i
    return outs, h
